# revision 16
# baseline (speedup 1.0000x reference)
"""Trainium2 Bass kernel for a transformer decoder layer (self-attn +
cross-attn + FFN), returning (out, attn1, attn2).

Sharding: 8 cores = 4 batches x 2 row-parities (core c -> batch c//2,
parity c%2 handles global query rows parity::2).  Every core runs the
same SPMD program; the causal structure is identical across cores at the
128-row-block level because query rows are parity-interleaved.
No collectives.  Matmuls run as float32r (full PE rate).
"""
import os
import sys

for _p in ('/opt/trn_rl_repo', '/root/.axon_site/_ro/trn_rl_repo'):
    if os.path.isdir(_p) and _p not in sys.path:
        sys.path.insert(0, _p)

import contextlib
import numpy as np
import ml_dtypes
import concourse.bass as bass
import concourse.mybir as mybir
from concourse.bass_utils import run_bass_kernel_spmd
from concourse.tile import TileContext
from concourse.masks import make_identity

FP = mybir.dt.float32
BF = mybir.dt.bfloat16
FR = mybir.dt.float32r
AF = mybir.ActivationFunctionType
ALU = mybir.AluOpType
AX = mybir.AxisListType

B, S, D, H, DH, DFF = 4, 1024, 1024, 16, 64, 4096
EPS = 1e-6
NCORES = 8
NEG = -8.0e9  # mask constant, pre-multiplied by 8 for the 1/8 logit scale


def _split_excess_waits(nc, max_waits=1):
    """This walrus build rejects >1 sem wait per instruction; hoist excess
    waits onto preceding same-engine NoOps."""
    for bb in nc.main_func.blocks:
        new_list = []
        for ins in bb.instructions:
            si = ins.sync_info
            if si is not None and si.on_wait and len(si.on_wait) > max_waits:
                waits = list(si.on_wait)
                extra, keep = waits[:-max_waits], waits[-max_waits:]
                k = 0
                while extra:
                    chunk, extra = extra[:max_waits], extra[max_waits:]
                    nop = mybir.InstNoOp(
                        name=f"{ins.name}-wsplit{k}",
                        opcode="NoOp",
                        engine=ins.engine,
                        sync_info=mybir.SyncInfo(on_wait=chunk, on_update=[]),
                        bass_nofuse=True,
                    )
                    nc.register_instruction(nop, overwrite=True)
                    new_list.append(nop)
                    k += 1
                ins.sync_info = mybir.SyncInfo(
                    on_wait=keep, on_update=list(si.on_update))
            new_list.append(ins)
        bb.instructions[:] = new_list


def build_program():
    nc = bass.Bass("TRN2", target_bir_lowering=False, debug=False,
                   num_devices=NCORES)

    dec = nc.dram_tensor("dec", [S, D], FP, kind="ExternalInput")
    decmy = nc.dram_tensor("decmy", [512, D], FP, kind="ExternalInput")
    enc = nc.dram_tensor("enc", [S, D], FP, kind="ExternalInput")
    m8d_in = nc.dram_tensor("m8diag", [4, 128, 512], BF, kind="ExternalInput")
    m8bt_in = nc.dram_tensor("m8bandT", [4, 2, 128, 128], BF,
                             kind="ExternalInput")
    wts = {}
    for nm in ('wq1', 'wk1', 'wv1', 'wq2', 'wk2', 'wv2'):
        wts[nm] = nc.dram_tensor(nm, [D, D], FR, kind="ExternalInput")
    for nm in ('wo1', 'wo2'):
        wts[nm] = nc.dram_tensor(nm, [D, D], BF, kind="ExternalInput")
    wts['w1'] = nc.dram_tensor("w1", [D, DFF], FR, kind="ExternalInput")
    ind2_in = nc.dram_tensor("ind2", [2, 128], FR, kind="ExternalInput")
    wts['w2'] = nc.dram_tensor("w2", [DFF, D], BF, kind="ExternalInput")

    out_p = nc.dram_tensor("out_p", [512, D], FP, kind="ExternalOutput")
    a1 = nc.dram_tensor("a1", [H, 512, S], FP, kind="ExternalOutput")
    a2 = nc.dram_tensor("a2", [H, 512, S], FP, kind="ExternalOutput")

    with TileContext(nc) as tc, contextlib.ExitStack() as ctx:
        const = ctx.enter_context(tc.tile_pool(name="const", bufs=1))
        rows = ctx.enter_context(tc.tile_pool(name="rows", bufs=2))
        smallT = ctx.enter_context(tc.tile_pool(name="smallT", bufs=2))
        work = ctx.enter_context(tc.tile_pool(name="work", bufs=2))
        ptp = ctx.enter_context(tc.tile_pool(name="ptp", bufs=2))
        o2p = ctx.enter_context(tc.tile_pool(name="o2p", bufs=1))
        misc = ctx.enter_context(tc.tile_pool(name="misc", bufs=2))
        wstream = ctx.enter_context(tc.tile_pool(name="wstream", bufs=2))
        ps_big = ctx.enter_context(
            tc.tile_pool(name="ps_big", bufs=4, space="PSUM"))
        ps_sm = ctx.enter_context(
            tc.tile_pool(name="ps_sm", bufs=2, space="PSUM"))
        ps_o = ctx.enter_context(
            tc.tile_pool(name="ps_o", bufs=2, space="PSUM"))

        ident = const.tile([128, 128], FP)
        make_identity(nc, ident)
        m8d = const.tile([128, 4, 512], BF)
        nc.sync.dma_start(out=m8d, in_=m8d_in.rearrange("i p c -> p i c"))
        m8bt = const.tile([128, 8, 128], BF)
        nc.sync.dma_start(
            out=m8bt, in_=m8bt_in.rearrange("i j p c -> p (i j) c"))
        epst = const.tile([128, 1], FP)
        nc.vector.memset(epst, EPS)
        ind2 = const.tile([2, 128], FR)
        nc.sync.dma_start(out=ind2, in_=ind2_in[:, :])

        # ------------------------ helpers ------------------------
        def transpose_to(dst, src_ap):
            tp = ps_sm.tile([128, 128], FP, tag="ps_sm")
            nc.tensor.transpose(tp, src_ap, ident)
            nc.scalar.copy(dst, tp)

        def load_rows_transpose(dst_featT, dram, nblk):
            for tb in range(nblk):
                for half in range(2):
                    rt = work.tile([128, 512], FP, tag="inrow")
                    nc.sync.dma_start(
                        out=rt, in_=dram[tb * 128:(tb + 1) * 128,
                                         half * 512:(half + 1) * 512])
                    for dd in range(4):
                        d = half * 4 + dd
                        transpose_to(
                            dst_featT[:, d, tb * 128:(tb + 1) * 128],
                            rt[:, dd * 128:(dd + 1) * 128])

        def proj_featT(dst, w_dram, rhs_featT, n_s):
            """dst[:, f, s] = sum_d w[d, f] * rhs_featT[d, s] (chunks)."""
            nsh = (n_s + 511) // 512
            for sh in range(nsh):
                sw = min(512, n_s - sh * 512)
                for fh in range(2):
                    pss = [ps_big.tile([128, 512], FP, tag="ps_proj", name=f"psp{j}")
                           for j in range(4)]
                    for d in range(8):
                        wt = wstream.tile([128, 512], FR, tag="wh")
                        nc.sync.dma_start(
                            out=wt, in_=w_dram[d * 128:(d + 1) * 128,
                                               fh * 512:(fh + 1) * 512])
                        for c in range(4):
                            nc.tensor.matmul(
                                pss[c][:, 0:sw],
                                wt[:, c * 128:(c + 1) * 128].bitcast(FR),
                                rhs_featT[:, d, sh * 512:sh * 512 + sw]
                                .bitcast(FR),
                                start=(d == 0), stop=(d == 7))
                    for c in range(4):
                        nc.scalar.copy(
                            dst[:, fh * 4 + c, sh * 512:sh * 512 + sw],
                            pss[c][:, 0:sw])

        def proj_tokmajor(dst, featT, w_dram):
            """dst[:, sb, :] = token-major projection x @ w (bf16 out)."""
            for g in range(2):
                for fh in range(2):
                    pss = [ps_big.tile([128, 512], FP, tag="ps_proj", name=f"psp{j}")
                           for j in range(4)]
                    for d in range(8):
                        wt = wstream.tile([128, 512], FR, tag="wh")
                        nc.sync.dma_start(
                            out=wt, in_=w_dram[d * 128:(d + 1) * 128,
                                               fh * 512:(fh + 1) * 512])
                        for j in range(4):
                            sb = g * 4 + j
                            nc.tensor.matmul(
                                pss[j],
                                featT[:, d, sb * 128:(sb + 1) * 128]
                                .bitcast(FR),
                                wt.bitcast(FR), start=(d == 0), stop=(d == 7))
                    for j in range(4):
                        sb = g * 4 + j
                        nc.scalar.copy(
                            dst[:, sb, fh * 512:(fh + 1) * 512], pss[j])

        def layernorm_inplace(rows_tile, i):
            x = rows_tile[:, i, :]
            st = misc.tile([128, 2, 6], FP, tag="bnst")
            nc.vector.bn_stats(st[:, 0, :], x[:, 0:512])
            nc.vector.bn_stats(st[:, 1, :], x[:, 512:1024])
            mv = misc.tile([128, 2], FP, tag="bnmv")
            nc.vector.bn_aggr(mv, st)
            std = misc.tile([128, 1], FP, tag="bnsd")
            nc.scalar.activation(std, mv[:, 1:2], AF.Sqrt, bias=epst[:, 0:1])
            rstd = misc.tile([128, 1], FP, tag="bnrs")
            nc.vector.reciprocal(rstd, std)
            nc.vector.tensor_scalar(
                out=x, in0=x, scalar1=mv[:, 0:1], scalar2=rstd,
                op0=ALU.subtract, op1=ALU.mult)

        def attention(qT, kT, v_sb, wo_dram, a_out, causal, resid_rows,
                      dst_rows, tag):
            o2T = o2p.tile([128, 8, 512], BF, tag="o2t")
            ot = None
            rr = None
            for h in range(H):
                c, hr = h // 2, (h % 2) * 64
                if h % 2 == 0:
                    rr = misc.tile([2, 512], FR, tag="rr")
                    srow = misc.tile([1, 512], FR, tag="srow")
                r4 = misc.tile([128, 4], FP, tag="r4")
                # ---- token-major pass: p, row sums, map output ----
                for i in range(4):
                    scd = (256 * (i + 1) - 1) // 512  # diag chunk index
                    nch = (scd + 1) if causal else 2
                    p = work.tile([128, 1024], FP, tag="p")
                    acc = misc.tile([128, 4], FP, tag="acc")
                    for k in range(nch):
                        zp = ps_big.tile([128, 512], FP, tag="ps_proj")
                        nc.tensor.matmul(
                            zp,
                            qT[hr:hr + 64, c, i * 128:(i + 1) * 128]
                            .bitcast(FR),
                            kT[hr:hr + 64, c, k * 512:(k + 1) * 512]
                            .bitcast(FR),
                            start=True, stop=True)
                        if causal and k == scd:
                            zm = work.tile([128, 512], FP, tag="zm")
                            nc.vector.tensor_tensor(
                                out=zm, in0=zp, in1=m8d[:, i, :], op=ALU.add)
                            src = zm
                        else:
                            src = zp
                        nc.scalar.activation(
                            p[:, k * 512:(k + 1) * 512], src, AF.Exp,
                            scale=0.125, accum_out=acc[:, k:k + 1])
                    if nch > 1:
                        ssum = misc.tile([128, 1], FP, tag="ssum")
                        nc.vector.reduce_sum(ssum, acc[:, 0:nch], axis=AX.X)
                    else:
                        ssum = acc[:, 0:1]
                    nc.vector.reciprocal(r4[:, i:i + 1], ssum)
                    w_i = 256 * (i + 1) if causal else 1024
                    nc.vector.tensor_scalar_mul(
                        p[:, 0:w_i], p[:, 0:w_i], r4[:, i:i + 1])
                    nc.sync.dma_start(
                        out=a_out[h, i * 128:(i + 1) * 128, 0:w_i],
                        in_=p[:, 0:w_i])
                # r4 -> row layout (partition 0) for the later broadcast
                for j in range(4):
                    rp = ps_sm.tile([128, 128], FP, tag="ps_sm",
                                    name=f"rp{j}")
                    nc.tensor.transpose(rp[0:1, 0:128], r4[:, j:j + 1], ident)
                    if h % 2 == 0:
                        nc.scalar.copy(rr[0:1, j * 128:(j + 1) * 128],
                                       rp[0:1, 0:128])
                    else:
                        nc.scalar.copy(srow[0:1, j * 128:(j + 1) * 128],
                                       rp[0:1, 0:128])
                # ---- feature-major pass: pT + AV ----
                if h % 2 == 0:
                    ot = ps_o.tile([128, 512], FP, tag="ps_o")
                for sb in range(8):
                    pT = ptp.tile([128, 512], BF, tag="pt")
                    for i in range(4):
                        lo, hi = i * 128, (i + 1) * 128
                        if causal and sb >= 2 * i + 2:
                            nc.vector.memset(pT[:, lo:hi], 0.0)
                            continue
                        zt = ps_sm.tile([128, 128], FP, tag="ps_sm")
                        nc.tensor.matmul(
                            zt,
                            kT[hr:hr + 64, c, sb * 128:(sb + 1) * 128]
                            .bitcast(FR),
                            qT[hr:hr + 64, c, i * 128:(i + 1) * 128]
                            .bitcast(FR),
                            start=True, stop=True)
                        if causal and sb >= 2 * i:
                            zmt = work.tile([128, 128], FP, tag="zmt")
                            nc.vector.tensor_tensor(
                                out=zmt, in0=zt,
                                in1=m8bt[:, i * 2 + (sb - 2 * i), :],
                                op=ALU.add)
                            src = zmt
                        else:
                            src = zt
                        nc.scalar.activation(
                            pT[:, lo:hi], src, AF.Exp, scale=0.125)
                    nc.tensor.matmul(
                        ot[hr:hr + 64, :],
                        v_sb[:, sb, h * 64:(h + 1) * 64], pT,
                        start=(sb == 0), stop=(sb == 7),
                        tile_position=(0, hr))
                if h % 2 == 1:
                    nc.sync.dma_start(out=rr[1:2, :], in_=srow[0:1, :])
                    bcp = ps_big.tile([128, 512], FP, tag="ps_proj")
                    nc.tensor.matmul(
                        bcp, ind2, rr, start=True, stop=True)
                    bc = misc.tile([128, 512], FP, tag="bc")
                    nc.scalar.copy(bc, bcp)
                    nc.vector.tensor_tensor(
                        out=o2T[:, c, :], in0=ot, in1=bc, op=ALU.mult)
            # ---- out projection + residual (into dst_rows) + LN ----
            for fh in range(2):
                pss = [ps_big.tile([128, 512], FP, tag="ps_proj", name=f"psp{j}")
                       for j in range(4)]
                for cc in range(8):
                    wt = wstream.tile([128, 512], BF, tag="wo")
                    nc.sync.dma_start(
                        out=wt, in_=wo_dram[cc * 128:(cc + 1) * 128,
                                            fh * 512:(fh + 1) * 512])
                    for i in range(4):
                        nc.tensor.matmul(
                            pss[i], o2T[:, cc, i * 128:(i + 1) * 128],
                            wt, start=(cc == 0), stop=(cc == 7))
                for i in range(4):
                    nc.vector.tensor_tensor(
                        out=dst_rows[:, i, fh * 512:(fh + 1) * 512],
                        in0=pss[i],
                        in1=resid_rows[:, i, fh * 512:(fh + 1) * 512],
                        op=ALU.add)
            for i in range(4):
                layernorm_inplace(dst_rows, i)

        # ================= phase 1: self-attention =================
        x_rows = rows.tile([128, 4, 1024], FP, tag="rows")
        for i in range(4):
            nc.sync.dma_start(out=x_rows[:, i, :],
                              in_=decmy[i * 128:(i + 1) * 128, :])
        xmyT = smallT.tile([128, 8, 512], FR, tag="smallT")
        for i in range(4):
            for d in range(8):
                transpose_to(xmyT[:, d, i * 128:(i + 1) * 128],
                             x_rows[:, i, d * 128:(d + 1) * 128])
        with tc.tile_pool(name="bigT", bufs=1) as bigT, \
             tc.tile_pool(name="ktp", bufs=1) as ktp, \
             tc.tile_pool(name="vp", bufs=1) as vp:
            xT = bigT.tile([128, 8, 1024], FR, tag="bigT")
            load_rows_transpose(xT, dec, 8)
            qT = smallT.tile([128, 8, 512], FR, tag="smallT")
            proj_featT(qT, wts['wq1'], xmyT, 512)
            kT = ktp.tile([128, 8, 1024], FR, tag="kt")
            proj_featT(kT, wts['wk1'], xT, 1024)
            v_sb = vp.tile([128, 8, 1024], BF, tag="v")
            proj_tokmajor(v_sb, xT, wts['wv1'])
            query = rows.tile([128, 4, 1024], FP, tag="rows")
            attention(qT, kT, v_sb, wts['wo1'], a1, True, x_rows, query, "s")

            # ================= phase 2: cross-attention =================
            encT = bigT.tile([128, 8, 1024], FR, tag="bigT")
            load_rows_transpose(encT, enc, 8)
            queryT = smallT.tile([128, 8, 512], FR, tag="smallT")
            for i in range(4):
                for d in range(8):
                    transpose_to(queryT[:, d, i * 128:(i + 1) * 128],
                                 query[:, i, d * 128:(d + 1) * 128])
            qT2 = smallT.tile([128, 8, 512], FR, tag="smallT")
            proj_featT(qT2, wts['wq2'], queryT, 512)
            kT2 = ktp.tile([128, 8, 1024], FR, tag="kt")
            proj_featT(kT2, wts['wk2'], encT, 1024)
            v2 = vp.tile([128, 8, 1024], BF, tag="v")
            proj_tokmajor(v2, encT, wts['wv2'])
            y = rows.tile([128, 4, 1024], FP, tag="rows")
            attention(qT2, kT2, v2, wts['wo2'], a2, False, query, y, "c")

        # ================= phase 3: FFN =================
        yT = smallT.tile([128, 8, 512], FR, tag="smallT")
        for i in range(4):
            for d in range(8):
                transpose_to(yT[:, d, i * 128:(i + 1) * 128],
                             y[:, i, d * 128:(d + 1) * 128])
        with tc.tile_pool(name="htp", bufs=1) as htp:
            hT = htp.tile([128, 32, 512], BF, tag="ht")
            for g in range(8):
                pss = [ps_big.tile([128, 512], FP, tag="ps_proj", name=f"psp{j}")
                       for j in range(4)]
                for d in range(8):
                    wt = wstream.tile([128, 512], FR, tag="wh")
                    nc.sync.dma_start(
                        out=wt, in_=wts['w1'][d * 128:(d + 1) * 128,
                                              g * 512:(g + 1) * 512])
                    for j in range(4):
                        nc.tensor.matmul(
                            pss[j],
                            wt[:, j * 128:(j + 1) * 128].bitcast(FR),
                            yT[:, d, :].bitcast(FR),
                            start=(d == 0), stop=(d == 7))
                for j in range(4):
                    nc.scalar.activation(hT[:, g * 4 + j, :], pss[j], AF.Relu)
            outrow = rows.tile([128, 4, 1024], FP, tag="rows")
            for fh in range(2):
                pss = [ps_big.tile([128, 512], FP, tag="ps_proj", name=f"psp{j}")
                       for j in range(4)]
                for dff in range(32):
                    wt = wstream.tile([128, 512], BF, tag="wo")
                    nc.sync.dma_start(
                        out=wt, in_=wts['w2'][dff * 128:(dff + 1) * 128,
                                              fh * 512:(fh + 1) * 512])
                    for i in range(4):
                        nc.tensor.matmul(
                            pss[i], hT[:, dff, i * 128:(i + 1) * 128],
                            wt, start=(dff == 0), stop=(dff == 31))
                for i in range(4):
                    nc.vector.tensor_tensor(
                        out=outrow[:, i, fh * 512:(fh + 1) * 512],
                        in0=pss[i],
                        in1=y[:, i, fh * 512:(fh + 1) * 512],
                        op=ALU.add)
            for i in range(4):
                layernorm_inplace(outrow, i)
                nc.sync.dma_start(out=out_p[i * 128:(i + 1) * 128, :],
                                  in_=outrow[:, i, :])
    _split_excess_waits(nc)
    return nc


_CACHE = {}


def _standard_structure(inputs):
    lam = np.asarray(inputs['look_ahead_mask'])
    pad = np.asarray(inputs['padding_mask'])
    if lam.shape != (1, 1, S, S) or pad.shape != (B, 1, 1, S):
        return False
    if not np.array_equal(lam[0, 0], np.triu(np.ones((S, S), np.float32), 1)):
        return False
    if np.any(pad != 0):
        return False
    for p in ('mha1', 'mha2'):
        for bn in ('qb', 'kb', 'vb', 'ob'):
            if np.any(np.asarray(inputs[f'{p}_{bn}']) != 0):
                return False
    if np.any(np.asarray(inputs['ffn_b1']) != 0) or \
       np.any(np.asarray(inputs['ffn_b2']) != 0):
        return False
    for i in (1, 2, 3):
        if np.any(np.asarray(inputs[f'ln{i}_g']) != 1):
            return False
        if np.any(np.asarray(inputs[f'ln{i}_b']) != 0):
            return False
    return True


def _numpy_fallback(inputs):
    f32 = np.float32
    inp = {k: np.asarray(v, f32) for k, v in inputs.items()}

    def split_heads(x):
        b, s, _ = x.shape
        return x.reshape(b, s, H, DH).transpose(0, 2, 1, 3)

    def mha(q_in, k_in, v_in, p, mask):
        q = split_heads(q_in @ inp[f'{p}_wq'] + inp[f'{p}_qb'])
        k = split_heads(k_in @ inp[f'{p}_wk'] + inp[f'{p}_kb'])
        v = split_heads(v_in @ inp[f'{p}_wv'] + inp[f'{p}_vb'])
        sc = np.einsum('bhqd,bhkd->bhqk', q, k) / np.sqrt(f32(DH))
        if mask is not None:
            sc = sc + mask * f32(-1e9)
        sc = sc - sc.max(-1, keepdims=True)
        e = np.exp(sc)
        attn = e / e.sum(-1, keepdims=True)
        o = np.einsum('bhqk,bhkd->bhqd', attn, v)
        o = o.transpose(0, 2, 1, 3).reshape(o.shape[0], -1, D)
        return o @ inp[f'{p}_wo'] + inp[f'{p}_ob'], attn.astype(f32)

    def ln(x, g, b):
        mu = x.mean(-1, keepdims=True)
        var = np.square(x - mu).mean(-1, keepdims=True)
        return (x - mu) / np.sqrt(var + EPS) * g + b

    x = inp['decoder_input']
    o1, attn1 = mha(x, x, x, 'mha1', inp['look_ahead_mask'])
    q = ln(x + o1, inp['ln1_g'], inp['ln1_b'])
    o2, attn2 = mha(q, inp['encoder_output'], inp['encoder_output'], 'mha2',
                    inp['padding_mask'])
    yy = ln(o2 + q, inp['ln2_g'], inp['ln2_b'])
    ffn = np.maximum(yy @ inp['ffn_w1'] + inp['ffn_b1'], 0) @ inp['ffn_w2'] \
        + inp['ffn_b2']
    out = ln(yy + ffn, inp['ln3_g'], inp['ln3_b'])
    return out.astype(f32), attn1, attn2


def make_in_maps(inputs):
    f32 = np.float32
    bf16 = ml_dtypes.bfloat16
    dec = np.ascontiguousarray(np.asarray(inputs['decoder_input'], f32))
    enc = np.ascontiguousarray(np.asarray(inputs['encoder_output'], f32))
    m8 = np.asarray(inputs['look_ahead_mask'], f32)[0, 0] * np.float32(NEG)

    shared = {
        'wq1': np.ascontiguousarray(np.asarray(inputs['mha1_wq'], f32)),
        'wk1': np.ascontiguousarray(np.asarray(inputs['mha1_wk'], f32)),
        'wv1': np.ascontiguousarray(np.asarray(inputs['mha1_wv'], f32)),
        'wo1': np.ascontiguousarray(np.asarray(inputs['mha1_wo']).astype(bf16)),
        'wq2': np.ascontiguousarray(np.asarray(inputs['mha2_wq'], f32)),
        'wk2': np.ascontiguousarray(np.asarray(inputs['mha2_wk'], f32)),
        'wv2': np.ascontiguousarray(np.asarray(inputs['mha2_wv'], f32)),
        'wo2': np.ascontiguousarray(np.asarray(inputs['mha2_wo']).astype(bf16)),
        'w1': np.ascontiguousarray(np.asarray(inputs['ffn_w1'], f32)),
        'w2': np.ascontiguousarray(np.asarray(inputs['ffn_w2']).astype(bf16)),
        'ind2': np.ascontiguousarray(np.repeat(np.eye(2, dtype=f32), 64,
                                               axis=1)),
    }

    in_maps = []
    for core in range(NCORES):
        b, par = core // 2, core % 2
        m8r = m8[par::2]  # [512, 1024] rows of my parity
        m8diag = np.zeros((4, 128, 512), f32)
        m8bandT = np.zeros((4, 2, 128, 128), f32)
        for i in range(4):
            scd = (256 * (i + 1) - 1) // 512
            m8diag[i] = m8r[i * 128:(i + 1) * 128, scd * 512:(scd + 1) * 512]
            for j in range(2):
                sb = 2 * i + j
                m8bandT[i, j] = m8r[i * 128:(i + 1) * 128,
                                    sb * 128:(sb + 1) * 128].T
        in_maps.append(dict(shared, dec=dec[b],
                            decmy=np.ascontiguousarray(dec[b][par::2]),
                            enc=enc[b], m8diag=m8diag.astype(bf16),
                            m8bandT=m8bandT.astype(bf16)))
    return in_maps


def assemble(results):
    f32 = np.float32
    out = np.zeros((B, S, D), f32)
    attn1 = np.zeros((B, H, S, S), f32)
    attn2 = np.zeros((B, H, S, S), f32)
    for core in range(NCORES):
        b, par = core // 2, core % 2
        r = results[core]
        out[b, par::2, :] = r['out_p']
        attn1[b, :, par::2, :] = r['a1']
        attn2[b, :, par::2, :] = r['a2']
    return out, attn1, attn2


def kernel(**inputs):
    if not _standard_structure(inputs):
        return _numpy_fallback(inputs)
    in_maps = make_in_maps(inputs)
    if 'nc' not in _CACHE:
        _CACHE['nc'] = build_program()
    res = run_bass_kernel_spmd(_CACHE['nc'], in_maps, list(range(NCORES)))
    return assemble(res.results)
